# revision 33
# baseline (speedup 1.0000x reference)
"""Trainium2 Bass kernel: batched Viterbi decode (Bert_BiLSTM_CRF).

Contract: kernel(feats [256,4096,6] f32, transitions [256,4096,6,6] f32)
returns (score [256] f32, path [256,4096] int32) — identical to the jax
reference (bit-exact forward recurrence, exact argmax backpointers with
first-max tie-break, exact integer backtracking).

Sharding: data-parallel over batch, 32 batches per core on 8 cores.

Per-core layout: time steps are grouped in windows of G=32 consecutive
steps; window w(t) = (t//G) % 4 selects partitions [32w, 32w+32), batch
is the within-window partition index. The sequential Viterbi chain runs
3 DVE instructions per step on [32, ·] operands; everything batched
(psi extraction, backward map-composition) then uses all 128 partitions.

Backward pass: path[t-1] = psi[t][path[t]] is associative function
composition over maps [6]->[6]; composed pairwise per 32-step run
(hidden under the forward chain), then a short boundary-tag scan and a
31-step parallel in-run chase reconstruct the full path exactly.
"""

import sys

import numpy as np

try:
    import concourse.bass  # noqa: F401
except ImportError:
    sys.path.insert(0, "/opt/trn_rl_repo")

B, T, K = 256, 4096, 6
KK = K * K
NCORES = 8
BC = B // NCORES  # 32 batches per core
START = 4  # tag_to_ix['[CLS]']
G = 32  # window run length (steps per partition-group run)
W = 4  # number of windows stacked in partitions
CH = W * G  # chain chunk length in steps
NEG = -10000.0

_CACHE: dict = {}


def _build(T_=T, BC_=BC, stages=("chain", "psi", "tree", "bwd")):
    import concourse.bacc as bacc
    import concourse.mybir as mybir
    from concourse.tile import TileContext

    f32 = mybir.dt.float32
    i32 = mybir.dt.int32
    Add = mybir.AluOpType.add
    Max = mybir.AluOpType.max
    Mult = mybir.AluOpType.mult
    Sub = mybir.AluOpType.subtract
    X = mybir.AxisListType.X
    # Pool has no compare/max ALU ops, so selections are arithmetic:
    # for d = x - y (==0 iff x==y, IEEE), (d*BIG)*BIG + v == v exactly at
    # d==0 and saturates hugely negative otherwise (double-mult covers
    # subnormal gaps); for integer-valued d, v - (d*d)*BIG does the same.
    BIG = 1e38

    NCH = T_ // CH  # chain chunks
    NRUN = T_ // G  # 32-step runs (backward chunks)
    HPW = NRUN // W  # runs per window (= NCH)
    TQ = T_ // W  # time steps per window

    nc = bacc.Bacc("TRN2", target_bir_lowering=False)
    trans_d = nc.dram_tensor("transitions", [BC_, T_, K, K], f32, kind="ExternalInput")
    feats_d = nc.dram_tensor("feats", [BC_, T_, K], f32, kind="ExternalInput")
    score_d = nc.dram_tensor("score", [BC_, 1], f32, kind="ExternalOutput")
    path_d = nc.dram_tensor("path", [BC_, T_], i32, kind="ExternalOutput")

    ve = nc.vector
    po = nc.gpsimd

    class _StagesDone(Exception):
        pass

    def win(t):
        return (t // G) % W

    with TileContext(nc) as tc:
        with (
            tc.tile_pool(name="persist", bufs=1) as pp,
            tc.tile_pool(name="ld", bufs=3) as ldp,
            tc.tile_pool(name="sc", bufs=2) as scp,
            tc.tile_pool(name="batch", bufs=2) as bp,
        ):
            # --- constants -------------------------------------------------
            iota_i = pp.tile([128, K], i32, tag="iota_i")
            po.iota(iota_i[:, :], pattern=[[1, K]], base=0, channel_multiplier=0)
            iota_f = pp.tile([128, K], f32, tag="iota_f")
            ve.tensor_copy(out=iota_f[:, :], in_=iota_i[:, :])
            # revj[j] = 6 - j: max over eq*revj picks the smallest tied j.
            revj = pp.tile([128, K], f32, tag="revj")
            ve.tensor_scalar(
                out=revj[:, :], in0=iota_f[:, :],
                scalar1=-1.0, scalar2=6.0, op0=Mult, op1=Add,
            )

            # --- persistent state ------------------------------------------
            delta = pp.tile([128, 2 * K], f32, tag="delta")  # ping-pong slots
            psi_rev = pp.tile([128, TQ * K], f32, tag="psi_rev")
            path_rev = pp.tile([128, TQ], f32, tag="path_rev")
            path_i = pp.tile([128, TQ], i32, tag="path_i")
            runmaps = pp.tile([128, HPW * K], f32, tag="runmaps")
            rm_flat = pp.tile([128, NRUN * K], f32, tag="rm_flat")
            bt = pp.tile([128, NRUN], f32, tag="bt")
            btw = pp.tile([128, HPW], f32, tag="btw")
            score_t = pp.tile([128, 1], f32, tag="score_t")
            lt = pp.tile([128, 1], f32, tag="lt")
            sml = pp.tile([128, 2 * K], f32, tag="sml")  # small scratch

            if len(stages) < 4:
                # Diagnostic partial builds: touch every persistent tile so
                # Tile's release pass sees an allocation for each.
                for tt in (delta, psi_rev, path_rev, runmaps, rm_flat, bt,
                           btw, score_t, lt, sml):
                    ve.memset(tt[:, :], 0.0)
                ve.memset(path_i[:, :], 0)

            # delta_0: -1e4 everywhere except START tag = 0. Lives at window
            # 0 partitions, slot 0.
            ve.memset(delta[0:32, 0:K], NEG)
            ve.memset(delta[0:32, START : START + 1], 0.0)

            psi3 = psi_rev.rearrange("p (s x) -> p s x", x=K)

            # Injection offsets within a chunk for the (up to) 12 batch
            # stages of the previous chunk: each Pool stage gets enough
            # chain steps before its DVE-reduce stage to finish.
            INJ_AT = (4, 44, 52, 68, 76, 88, 94, 102, 108, 114, 118, 124)

            def emit_chunk(h, stg=()):
                """DMA + exact sequential chain for chunk h, injecting the
                previous chunk's batched stages at INJ_AT offsets."""
                inj_at = list(INJ_AT[: len(stg)])
                t0 = h * CH
                tr_t = ldp.tile([128, G * KK], f32, tag="tr")
                ft_t = ldp.tile([128, G * K], f32, tag="ft")
                for w in range(W):
                    ta, tb = t0 + w * G, t0 + (w + 1) * G
                    nc.sync.dma_start(
                        out=tr_t[32 * w : 32 * (w + 1), :],
                        in_=trans_d[:, ta:tb, :, :],
                    )
                    nc.sync.dma_start(
                        out=ft_t[32 * w : 32 * (w + 1), :],
                        in_=feats_d[:, ta:tb, :],
                    )

                sc_t = scp.tile([128, G * KK], f32, tag="scores")
                m_t = scp.tile([128, G * K], f32, tag="m")
                if h == 0:
                    # t=0 has no chain step; give its scores/m slot defined
                    # junk so the full-range psi batch below is legal. The
                    # resulting psi_rev[t=0] is overwritten with the
                    # identity map right after.
                    ve.memset(sc_t[0:32, 0:KK], 0.0)
                    ve.memset(m_t[0:32, 0:K], 0.0)
                sc4 = sc_t.rearrange("p (r i j) -> p r i j", i=K, j=K)
                tr4 = tr_t.rearrange("p (r i j) -> p r i j", i=K, j=K)
                m3 = m_t.rearrange("p (r i) -> p r i", i=K)
                f3 = ft_t.rearrange("p (r i) -> p r i", i=K)

                # Dual-SBUF-input ops must share a partition base (walrus
                # NCC_IBIR297), so delta_t is written by step t directly
                # into step t+1's window; every read is then window-local.
                for t in (range(max(1, t0), t0 + CH) if "chain" in stages else []):
                    if (t - t0) in inj_at:
                        stg[inj_at.index(t - t0)]()
                    w, r = win(t), t % G
                    nw = win(t + 1) if t + 1 < T_ else w
                    sl, psl = (t % 2) * K, ((t - 1) % 2) * K
                    WS = slice(32 * w, 32 * (w + 1))
                    NS = slice(32 * nw, 32 * (nw + 1))
                    db = (
                        delta[WS, psl : psl + K]
                        .unsqueeze(1)
                        .broadcast_to([32, K, K])
                    )
                    ve.tensor_tensor(
                        out=sc4[WS, r], in0=tr4[WS, r], in1=db, op=Add
                    )
                    ve.tensor_reduce(out=m3[WS, r], in_=sc4[WS, r], axis=X, op=Max)
                    ve.tensor_tensor(
                        out=delta[NS, sl : sl + K],
                        in0=m3[WS, r],
                        in1=f3[WS, r],
                        op=Add,
                    )
                return sc_t, m_t

            def batch_stages(h, sc_t, m_t):
                """Closures for chunk h's batched work, split so every DVE
                reduce is a separate stage from the Pool work it consumes.
                The caller spaces stages along the next chunk's chain; the
                chain steps between a Pool stage and its reduce cover the
                Pool latency, so the in-order DVE never waits on Pool."""
                if "psi" not in stages:
                    return []
                st = {}

                def s_psi_pool():
                    eq_t = bp.tile([128, G * KK], f32, tag="eq")
                    tm_t = bp.tile([128, G * KK], f32, tag="tm")
                    scn = sc_t.rearrange("p (r i j) -> p r i j", i=K, j=K)
                    mb = (
                        m_t.rearrange("p (r i) -> p r i", i=K)
                        .unsqueeze(3)
                        .broadcast_to([128, G, K, K])
                    )
                    eqn = eq_t.rearrange("p (r i j) -> p r i j", i=K, j=K)
                    tmn = tm_t.rearrange("p (r i j) -> p r i j", i=K, j=K)
                    rvb = (
                        revj.unsqueeze(1).unsqueeze(1).broadcast_to([128, G, K, K])
                    )
                    po.tensor_tensor(out=eqn, in0=scn, in1=mb, op=Sub)
                    po.tensor_scalar(
                        out=tmn, in0=eqn, scalar1=BIG, scalar2=BIG,
                        op0=Mult, op1=Mult,
                    )
                    po.tensor_tensor(out=eqn, in0=tmn, in1=rvb, op=Add)
                    st["eqn"] = eqn

                def s_psi_red():
                    ve.tensor_reduce(
                        out=psi3[:, h * G : (h + 1) * G],
                        in_=st["eqn"], axis=X, op=Max,
                    )
                    if h == 0:
                        # psi at t=0 is never produced by the chain; the
                        # backward pass uses M'_0 only in run 0's tree,
                        # where the identity map (rev-coded 6-x) is correct.
                        ve.tensor_copy(out=psi_rev[0:32, 0:K], in_=revj[0:32, :])

                out = [s_psi_pool, s_psi_red]
                if "tree" not in stages:
                    return out
                # Per-run composition trees (runs c = 4h+w): levels of
                # 16, 8, 4, 2, 1 pairwise compositions; Q = earlier o later
                # via cand[x,j] = E[j] - (L[x]-revj[j])^2*BIG, segmented max.
                st["src"] = psi_rev[:, h * G * K : (h + 1) * G * K]

                def s_lvl_pool(npair):
                    s3 = st["src"].rearrange("p (q x) -> p q x", x=K)
                    later = (
                        s3[:, 1 : 2 * npair : 2, :]
                        .unsqueeze(3)
                        .broadcast_to([128, npair, K, K])
                    )
                    earlier = (
                        s3[:, 0 : 2 * npair : 2, :]
                        .unsqueeze(2)
                        .broadcast_to([128, npair, K, K])
                    )
                    rvb2 = (
                        revj.unsqueeze(1)
                        .unsqueeze(1)
                        .broadcast_to([128, npair, K, K])
                    )
                    eqc = bp.tile([128, npair * KK], f32, tag="eqc")
                    sel = bp.tile([128, npair * KK], f32, tag="sel")
                    e4 = eqc.rearrange("p (q x j) -> p q x j", x=K, j=K)
                    l4 = sel.rearrange("p (q x j) -> p q x j", x=K, j=K)
                    po.tensor_tensor(out=e4, in0=later, in1=rvb2, op=Sub)
                    po.tensor_tensor(out=l4, in0=e4, in1=e4, op=Mult)
                    po.tensor_scalar(
                        out=e4, in0=l4, scalar1=-BIG, scalar2=0.0,
                        op0=Mult, op1=Add,
                    )
                    po.tensor_tensor(out=l4, in0=e4, in1=earlier, op=Add)
                    st["l4"] = l4

                def s_lvl_red(npair):
                    if npair > 1:
                        dst = bp.tile([128, npair * K], f32, tag="lvl")
                        d3 = dst.rearrange("p (q x) -> p q x", x=K)
                    else:
                        dst = None
                        d3 = runmaps.rearrange("p (q x) -> p q x", x=K)[
                            :, h : h + 1, :
                        ]
                    ve.tensor_reduce(out=d3, in_=st["l4"], axis=X, op=Max)
                    st["src"] = dst

                npair = G // 2
                while npair >= 1:
                    out.append(lambda n=npair: s_lvl_pool(n))
                    out.append(lambda n=npair: s_lvl_red(n))
                    npair //= 2
                return out

            # Drive: chunk h's chain with chunk h-1's batched stages
            # injected at spaced points, one stage per 16 chain steps —
            # each stage's Pool half runs while the chain covers the gap
            # to the next stage's DVE reduce.
            prev = None
            for h in range(NCH):
                stg = batch_stages(*prev) if prev is not None else []
                cur = emit_chunk(h, stg)
                prev = (h, *cur)
            for s in batch_stages(*prev):
                s()
            # --- final score / last tag (rev-coded) ------------------------
            wf = win(T_ - 1)
            FS = slice(32 * wf, 32 * (wf + 1))
            slf = ((T_ - 1) % 2) * K
            df = delta[FS, slf : slf + K]
            ve.tensor_reduce(out=score_t[FS, 0:1], in_=df, axis=X, op=Max)
            ve.tensor_tensor(
                out=sml[FS, 0:K],
                in0=df,
                in1=score_t[FS, 0:1].broadcast_to([32, K]),
                op=Sub,
            )
            ve.tensor_scalar(
                out=sml[FS, K : 2 * K], in0=sml[FS, 0:K],
                scalar1=BIG, scalar2=BIG, op0=Mult, op1=Mult,
            )
            ve.tensor_tensor(
                out=sml[FS, 0:K], in0=sml[FS, K : 2 * K], in1=revj[FS, :], op=Add
            )
            ve.tensor_reduce(out=lt[FS, 0:1], in_=sml[FS, 0:K], axis=X, op=Max)

            # --- boundary-tag scan over runs (right to left) ---------------
            if "bwd" not in stages:
                nc.sync.dma_start(out=score_d[:, :], in_=score_t[FS, 0:1])
                nc.sync.dma_start(out=path_d[:, 0:TQ], in_=path_i[0:BC_, :])
                raise _StagesDone
            # Collapse per-window runmaps into batch-only partitions first.
            rmf3 = rm_flat.rearrange("p (c x) -> p c x", x=K)
            for w in range(W):
                nc.sync.dma_start(
                    out=rmf3[0:32, w::W, :],
                    in_=runmaps[32 * w : 32 * (w + 1), :].rearrange(
                        "p (h x) -> p h x", x=K
                    ),
                )
            nc.sync.dma_start(out=bt[0:32, NRUN - 1 : NRUN], in_=lt[FS, 0:1])
            eqb = sml  # reuse small scratch at window-0 partitions
            for c in range(NRUN - 1, 0, -1):
                ve.tensor_tensor(
                    out=eqb[0:32, 0:K],
                    in0=bt[0:32, c : c + 1].broadcast_to([32, K]),
                    in1=revj[0:32, :],
                    op=Sub,
                )
                ve.tensor_tensor(
                    out=eqb[0:32, K : 2 * K],
                    in0=eqb[0:32, 0:K],
                    in1=eqb[0:32, 0:K],
                    op=Mult,
                )
                ve.scalar_tensor_tensor(
                    out=eqb[0:32, 0:K],
                    in0=eqb[0:32, K : 2 * K],
                    scalar=-BIG,
                    in1=rm_flat[0:32, c * K : (c + 1) * K],
                    op0=Mult,
                    op1=Add,
                )
                ve.tensor_reduce(
                    out=bt[0:32, c - 1 : c], in_=eqb[0:32, 0:K], axis=X, op=Max
                )
            # Scatter boundary tags back to their runs' windows.
            for w in range(W):
                nc.sync.dma_start(
                    out=btw[32 * w : 32 * (w + 1), :], in_=bt[0:32, w::W]
                )

            # --- in-run chase: 31 parallel steps over all runs -------------
            pr3 = path_rev.rearrange("p (h r) -> p h r", r=G)
            ve.tensor_copy(out=pr3[:, :, G - 1], in_=btw[:, :])
            eq3_t = bp.tile([128, HPW * K], f32, tag="eq3")
            sel3_t = bp.tile([128, HPW * K], f32, tag="sel3")
            e3 = eq3_t.rearrange("p (h x) -> p h x", x=K)
            s3b = sel3_t.rearrange("p (h x) -> p h x", x=K)
            rvb3 = revj.unsqueeze(1).broadcast_to([128, HPW, K])
            for r in range(G - 1, 0, -1):
                ve.tensor_tensor(
                    out=e3,
                    in0=pr3[:, :, r].unsqueeze(2).broadcast_to([128, HPW, K]),
                    in1=rvb3,
                    op=Sub,
                )
                ve.tensor_tensor(out=s3b, in0=e3, in1=e3, op=Mult)
                ve.scalar_tensor_tensor(
                    out=e3, in0=s3b, scalar=-BIG, in1=psi3[:, r::G, :],
                    op0=Mult, op1=Add,
                )
                ve.tensor_reduce(out=pr3[:, :, r - 1], in_=e3, axis=X, op=Max)

            # --- decode (path = 6 - rev), cast, store ----------------------
            ve.tensor_scalar(
                out=path_i[:, :], in0=path_rev[:, :],
                scalar1=-1.0, scalar2=6.0, op0=Mult, op1=Add,
            )
            pd3 = path_d.rearrange("b (h x) -> b h x", x=CH)
            pi3 = path_i.rearrange("p (h r) -> p h r", r=G)
            for w in range(W):
                nc.sync.dma_start(
                    out=pd3[:, :, w * G : (w + 1) * G],
                    in_=pi3[32 * w : 32 * (w + 1), :, :],
                )
            nc.sync.dma_start(out=score_d[:, :], in_=score_t[FS, 0:1])

    nc.compile()
    return nc


def _get_nc(T_=T, BC_=BC):
    key = (T_, BC_)
    if key not in _CACHE:
        _CACHE[key] = _build(T_, BC_)
    return _CACHE[key]


def kernel(feats: np.ndarray, transitions: np.ndarray):
    from concourse.bass_utils import run_bass_kernel_spmd

    feats = np.ascontiguousarray(feats, dtype=np.float32)
    transitions = np.ascontiguousarray(transitions, dtype=np.float32)
    nc = _get_nc()
    in_maps = [
        {
            "feats": feats[i * BC : (i + 1) * BC],
            "transitions": transitions[i * BC : (i + 1) * BC],
        }
        for i in range(NCORES)
    ]
    res = run_bass_kernel_spmd(nc, in_maps, core_ids=list(range(NCORES)))
    score = np.concatenate([r["score"][:, 0] for r in res.results])
    path = np.concatenate([r["path"] for r in res.results]).astype(np.int32)
    return score, path
